# revision 15
# baseline (speedup 1.0000x reference)
"""FlowNetC correlation (max_disp=20, stride2=2, kernel_size=1) on 8 trn2 cores.

Strategy (pure data parallel over batch, 2 batches per core):
  out[b, d, y, x] = (1/256) * <in1[b,:,y,x], in2[b,:,y+dy,x+dx]>,
  dy = 2*(d//21) - 20, dx = 2*(d%21) - 20.

Displacements are even, so (y,x) only pairs with (y2,x2) of equal parity
per axis.  The host pre-shards inputs into parity-quadrant layouts
(y = py + 2*y', x = px + 2*x'):
  in1r[b, ch, p, (py, px, y', x')], in2r[b, ch, p, (py, px, y2', x2')]
Per batch, 24 stationary groups (py, blk, px) each hold 128 in1 pixels
(4 same-parity rows x 32 same-parity cols); the moving operand is the
same-parity in2 rows those pixels can see (clipped to the image).  The
TensorEngine computes a [128, <=768] Gram block per group (C=256
contraction = 2 accumulated chunks, fp32r at full PE rate), scaled by 1/C
and dumped contiguously to DRAM scratch; rows outside the image are never
computed and stay zero via the pre-zeroed output buffer.  The diagonal
extraction (deskew) to [B, 441, 48, 64] is a strided-view gather on the
host — flat DRAM addressing makes the Gram diagonals plain strided
slices, which on-chip 2D SBUF addressing cannot express.
"""

import numpy as np
from contextlib import ExitStack

import concourse.tile as tile
from concourse import bacc, mybir
import concourse.bass_utils as bass_utils

B, C, H, W = 16, 256, 48, 64
GRID = 21          # displacement steps per axis
N_CORES = 8
B_LOC = B // N_CORES   # 2 batches per core
CHUNKS = C // 128      # 2 contraction chunks
PY, PX, BLK, R, T = 2, 2, 6, 4, 24
NGROUPS = PY * BLK * PX        # 24 groups per batch
BOX_F = T * 32                 # 768 free elems per Gram block slot
_compiled = None


def _t_range(blk):
    """Valid t-range for a y-block: t = r + dy_s, in2 parity-row index
    y2' = 4*blk + t - 10 must lie in [0, 24)."""
    t0 = max(0, 10 - 4 * blk)
    t1 = min(T, 34 - 4 * blk)
    return t0, t1


def _build():
    nc = bacc.Bacc("TRN2", target_bir_lowering=False, debug=False,
                   num_devices=N_CORES)
    in1_d = nc.dram_tensor("in1r", [B_LOC, CHUNKS, 128, PY * PX * T * 32],
                           mybir.dt.float32r, kind="ExternalInput").ap()
    in2_d = nc.dram_tensor("in2r", [B_LOC, CHUNKS, 128, PY * PX * T * 32],
                           mybir.dt.float32r, kind="ExternalInput").ap()
    scr_d = nc.dram_tensor("scratch", [B_LOC, PY * BLK, 128, PX * BOX_F],
                           mybir.dt.float32, kind="ExternalOutput").ap()

    inv_c = 1.0 / C
    f32, f32r = mybir.dt.float32, mybir.dt.float32r

    with tile.TileContext(nc) as tc, ExitStack() as ctx:
        in1_pool = ctx.enter_context(tc.tile_pool(name="in1", bufs=2))
        in2_pool = ctx.enter_context(tc.tile_pool(name="in2", bufs=2))
        psum_pool = ctx.enter_context(
            tc.tile_pool(name="psum", bufs=4, space="PSUM"))
        stage_pool = ctx.enter_context(tc.tile_pool(name="stage", bufs=6))

        QW = T * 32                       # quadrant width (768)
        in1_all, in2_all = {}, {}
        # All loads (both batches) emitted up front: they have no deps, so
        # they never queue behind store waits on the SP sequencer FIFO.
        for b in range(B_LOC):
            for qi in range(PY * PX):
                for ch in range(CHUNKS):
                    t1 = in1_pool.tile([128, QW], f32r,
                                       tag=f"in1_{ch}_{qi}")
                    nc.sync.dma_start(
                        t1[:], in1_d[b, ch, :, qi * QW:(qi + 1) * QW])
                    in1_all[b, ch, qi] = t1
                    t2 = in2_pool.tile([128, QW], f32r,
                                       tag=f"in2_{ch}_{qi}")
                    nc.sync.dma_start(
                        t2[:], in2_d[b, ch, :, qi * QW:(qi + 1) * QW])
                    in2_all[b, ch, qi] = t2
        for b in range(B_LOC):
            in1_t = {k[1:]: v for k, v in in1_all.items() if k[0] == b}
            in2_t = {k[1:]: v for k, v in in2_all.items() if k[0] == b}

            for py in range(PY):
                for blk in range(BLK):
                    gp = py * BLK + blk
                    t0, t1r = _t_range(blk)
                    vn = (t1r - t0) * 32          # valid Gram cols
                    # psum placement: single <=512 matmul left-aligned,
                    # else right-aligned so both bank tiles are >=256.
                    if vn <= 512:
                        tiles = [(0, vn)]
                    else:
                        tiles = [(BOX_F - vn, 512), (512, BOX_F)]
                    p_lo, p_hi = tiles[0][0], tiles[-1][1]
                    # Both x-parities of a (py, blk) pair share one stage
                    # tile and one store (px folded into the free dim).
                    stage = stage_pool.tile([128, PX * BOX_F], f32)
                    for px in range(PX):
                        qi = py * PX + px
                        psum = psum_pool.tile([128, BOX_F], f32)
                        for (n_lo, n_hi) in tiles:
                            for ch in range(CHUNKS):
                                lhsT = in1_t[ch, qi][
                                    :, blk * 128:blk * 128 + 128]
                                # in2 cols for psum range [n_lo, n_hi):
                                # y2' = 4*blk + t - 10, t = t0 + (n - p_lo)/32
                                c_lo = (4 * blk - 10 + t0) * 32 + (n_lo - p_lo)
                                rhs = in2_t[ch, qi][
                                    :, c_lo:c_lo + (n_hi - n_lo)]
                                nc.tensor.matmul(
                                    psum[:, n_lo:n_hi], lhsT, rhs,
                                    start=(ch == 0), stop=(ch == CHUNKS - 1))
                        # Split the scaled PSUM->SBUF copy so ACT (0.83ns/el,
                        # 172cyc setup) and DVE (1.04ns/el, 120cyc setup)
                        # finish together.
                        mid = p_lo + min(p_hi - p_lo,
                                         int(0.556 * (p_hi - p_lo) + 81))
                        o = px * BOX_F
                        nc.scalar.mul(stage[:, o + p_lo:o + mid],
                                      psum[:, p_lo:mid], inv_c)
                        if mid < p_hi:
                            nc.vector.tensor_scalar_mul(
                                stage[:, o + mid:o + p_hi],
                                psum[:, mid:p_hi], inv_c)
                    dst = scr_d[b, gp].rearrange(
                        "p (px n) -> p px n", px=PX)[:, :, t0 * 32:t0 * 32 + vn]
                    src = stage[:].rearrange(
                        "p (px n) -> p px n", px=PX)[:, :, p_lo:p_hi]
                    nc.sync.dma_start(dst, src)
    nc.finalize()
    return nc


def _shard_inputs(input1: np.ndarray, input2: np.ndarray):
    """Full inputs -> parity-quadrant layouts (B, CHUNKS, 128, 4*24*32)."""
    # (B, C, H, W) -> (B, C, py, px, y', x')
    a1 = input1.reshape(B, C, H // 2, 2, W // 2, 2).transpose(0, 1, 3, 5, 2, 4)
    in1r = np.ascontiguousarray(a1).reshape(B, CHUNKS, 128, PY * PX * T * 32)
    a2 = input2.reshape(B, C, H // 2, 2, W // 2, 2).transpose(0, 1, 3, 5, 2, 4)
    in2r = np.ascontiguousarray(a2).reshape(B, CHUNKS, 128, PY * PX * T * 32)
    return in1r, in2r


def _deskew(scratch_b: np.ndarray) -> np.ndarray:
    """(PY*BLK, 128, PX*BOX_F) Gram blocks of one batch -> (441, 48, 64)."""
    flat = np.ascontiguousarray(scratch_b).ravel()
    total = flat.size
    buf = np.zeros(10 + total + 1024, dtype=np.float32)
    buf[10:10 + total] = flat
    sz = buf.itemsize
    # flat index of (dy_s, dxi, py, blk, r, px, x') with
    #   gp = py*BLK+blk, m = 32r+x', t = r+dy_s, j = x'+dxi-10:
    #   gp*(128*PX*BOX_F) + m*(PX*BOX_F) + px*BOX_F + t*32 + j
    GS2 = 128 * PX * BOX_F
    strides = (32, 1, BLK * GS2, GS2, 32 * PX * BOX_F + 32, BOX_F,
               PX * BOX_F + 1)
    view = np.lib.stride_tricks.as_strided(
        buf,
        shape=(GRID, GRID, PY, BLK, R, PX, 32),
        strides=tuple(s * sz for s in strides))
    # -> (dy_s, dxi, blk, r, py, x', px): (blk,r,py)->y, (x',px)->x
    out5 = view.transpose(0, 1, 3, 4, 2, 6, 5).reshape(GRID, GRID, H, W)
    dxe = 2 * np.arange(GRID) - 20
    xs = np.arange(W)
    ok = ((xs[None, :] + dxe[:, None] >= 0)
          & (xs[None, :] + dxe[:, None] < W)).astype(np.float32)
    out5 = out5 * ok[None, :, None, :]
    return out5.reshape(GRID * GRID, H, W)


def kernel(input1: np.ndarray, input2: np.ndarray) -> np.ndarray:
    global _compiled
    if _compiled is None:
        _compiled = _build()
    nc = _compiled

    input1 = np.ascontiguousarray(input1, dtype=np.float32)
    input2 = np.ascontiguousarray(input2, dtype=np.float32)
    in1r, in2r = _shard_inputs(input1, input2)
    in_maps = [
        {"in1r": np.ascontiguousarray(in1r[k * B_LOC:(k + 1) * B_LOC]),
         "in2r": np.ascontiguousarray(in2r[k * B_LOC:(k + 1) * B_LOC])}
        for k in range(N_CORES)
    ]
    res = None
    for attempt in range(3):
        try:
            res = bass_utils.run_bass_kernel_spmd(
                nc, in_maps, core_ids=list(range(N_CORES)))
            break
        except Exception:
            # Transient NRT/axon device errors recover on retry.
            if attempt == 2:
                raise
            import time as _time
            _time.sleep(2.0)
    out = np.empty((B, GRID * GRID, H, W), dtype=np.float32)
    for k in range(N_CORES):
        scr = res.results[k]["scratch"]
        for b in range(B_LOC):
            out[k * B_LOC + b] = _deskew(scr[b])
    return out
